# revision 22
# baseline (speedup 1.0000x reference)
"""Trainium2 Bass kernel for ConditionalLinearAttention (v6).

Math (per batch element b, shapes hardcoded):
  xf  = x[b].reshape(256, 4096)
  cf  = cond_emb[b].reshape(512, 128)
  kv  = Wcond @ cf                      # (1024, 128)
  k   = softmax(kv[:512], per-row over the 128 cond positions)
  v   = kv[512:]
  ctx[h] = k_h @ v_h.T                  # (64, 64) per head h
  out = Wout @ apply(ctx) @ Wq @ xf + b_out

ctx is tiny and per-batch, so the whole attention folds into one per-batch
matrix W_comb = Wout @ ctxE @ Wq (256x256); the spatial dimension then sees
ONE (256x256)@(256x4096) GEMM. Sharding: data-parallel over batch, one
batch element per core.

v6 design notes (lessons from v2-v5 ntff profiles):
  * critical path = weight stream -> phase-1 chain -> W_comb -> 7.7us of
    phase-2 matmuls -> out stream; everything else overlaps it.
  * cross-engine handoffs cost 0.3-1us each and DMA completion semaphores
    trail the last byte by 0.7-2us, so: few large weight transfers (one
    semaphore each, v2-proven shapes), the k-half matmuls gated on the
    FIRST transfer only, and the 1/Z softmax scale folded into per-pair Wq
    slices (rc indexes the contraction dim of the A matmul, so it can
    scale either operand) - the diagonal-block extraction then runs as
    plain copies in parallel with the reciprocal.
  * W_comb accumulation is interleaved with A production, reusing the dead
    kv PSUM slots, so W_comb trails the last A chunk by one matmul.
  * the PE HAM clock gate needs ~3.4us of sustained activity to reach
    2.4GHz and re-throttles after idle windows.  Junk matmuls (N=128,
    ~107ns) cover the head; later groups are ANCHORED on phase-1 tiles
    (expkT, A_sb) so the Tile scheduler cannot hoist them early - they
    become ready exactly when the gap they must bridge opens.
  * SWDGE (gpsimd) input DMA was tried and abandoned: its completion sem
    trails data by ~1.5us/transfer (engine-7/15 ring contention).  int8
    x with cast-during-DMA was tried and abandoned: SDMA paces on the
    2-byte SBUF side, so it saves no stream time and costs accuracy.
"""

import os

import numpy as np

B = 8
C = 256
N_SPATIAL = 4096  # 64*64
P = 128
N_CORES = 8

WARM = int(os.environ.get("KERNEL_WARM", "34"))  # PE warmup matmuls at head
G1 = int(os.environ.get("KERNEL_G1", "6"))   # anchored on expkT
G2 = int(os.environ.get("KERNEL_G2", "4"))   # anchored on A_sb[:, 1]
G3 = int(os.environ.get("KERNEL_G3", "8"))   # anchored on A_sb[:, 3]

_CACHE = {}
LAST_RESULTS = None  # BassKernelResults of the most recent run (for test.py)


def _build_nc():
    import concourse.bacc as bacc
    import concourse.mybir as mybir
    import concourse.tile as tile

    fp32 = mybir.dt.float32
    bf16 = mybir.dt.bfloat16
    AF = mybir.ActivationFunctionType

    nc = bacc.Bacc("TRN2", target_bir_lowering=False, debug=False,
                   num_devices=N_CORES)

    # p1:    cf (4x128 cols) + wck (4x512)                  -> [128, 2560]
    # p2:    wcv (4x512)                                    -> [128, 2048]
    # p3:    wq_perm (4x256) + woT_perm (4x256)             -> [128, 2048]
    # xp:    x interleaved c=2p+ck, 4 col-chunks of 1024    -> [128,4,2,1024]
    # bias2: b_out[2p+mo]                                   -> [128, 2] f32
    # outp:  out rows 2p+mo, 8 col-chunks of 512            -> [128,8,2,512]
    p1_t = nc.dram_tensor("p1", [P, 2560], bf16, kind="ExternalInput").ap()
    p2_t = nc.dram_tensor("p2", [P, 2048], bf16, kind="ExternalInput").ap()
    p3_t = nc.dram_tensor("p3", [P, 2048], bf16, kind="ExternalInput").ap()
    xp_t = nc.dram_tensor("xp", [P, 4, 2, 1024], bf16, kind="ExternalInput").ap()
    bias_t = nc.dram_tensor("bias2", [P, 2], fp32, kind="ExternalInput").ap()
    out_t = nc.dram_tensor("out", [P, 8, 2, 512], bf16, kind="ExternalOutput").ap()

    with tile.TileContext(nc) as tc:
        with (
            tc.tile_pool(name="main", bufs=1) as mainp,
            tc.tile_pool(name="work", bufs=2) as workp,
            tc.tile_pool(name="outp", bufs=6) as outp,
            tc.tile_pool(name="ps", bufs=2, space="PSUM") as psp,
            tc.tile_pool(name="psA", bufs=3, space="PSUM") as psA,
            tc.tile_pool(name="psO", bufs=3, space="PSUM") as psO,
        ):
            # --- junk-matmul operand, first so the PE can warm immediately
            wl = mainp.tile([P, 128], bf16)
            nc.vector.memset(wl, 0.0)

            # --- input DMA triggers, critical-path order, all on sync HWDGE
            p1_sb = mainp.tile([P, 2560], bf16)
            nc.sync.dma_start(p1_sb, p1_t)
            p2_sb = mainp.tile([P, 2048], bf16)
            nc.sync.dma_start(p2_sb, p2_t)
            p3_sb = mainp.tile([P, 2048], bf16)
            nc.sync.dma_start(p3_sb, p3_t)
            x_sb = []
            for cc in range(4):
                t = mainp.tile([P, 2, 1024], bf16, tag=f"x{cc}")
                nc.sync.dma_start(t, xp_t[:, cc, :, :])
                x_sb.append(t)
            # small stuff off the sync ring
            bias_sb = mainp.tile([P, 2], fp32)
            nc.gpsimd.dma_start(bias_sb, bias_t)

            # persistent SBUF tiles + fills (not on gpsimd)
            vTo = mainp.tile([P, 4, 129], bf16)
            nc.vector.memset(vTo[:, :, 128:129], 1.0)
            ctx_bd = mainp.tile([P, 4, 128], bf16)
            nc.vector.memset(ctx_bd, 0.0)

            def keep_warm(n, op=None, name="pj"):
                src = op if op is not None else wl
                for _ in range(n):
                    pj = psO.tile([P, 512], fp32, tag="O", name=name)
                    nc.tensor.matmul(pj[:, 0:128], src, src, start=True,
                                     stop=True)

            keep_warm(WARM)

            # --- phase 1: per-batch W_comb (256x256) ---
            # kvT k-half (gated on p1's single semaphore), then v-half
            # (gated on p2) while exp() runs on the scalar engine.
            pkv = psp.tile([P, 512], fp32, tag="p1")
            for j in range(4):
                nc.tensor.matmul(pkv, p1_sb[:, 128 * j:128 * (j + 1)],
                                 p1_sb[:, 512 + 512 * j:512 + 512 * (j + 1)],
                                 start=(j == 0), stop=(j == 3))
            pvv = psp.tile([P, 4, 128], fp32, tag="p1")
            for j in range(4):
                nc.tensor.matmul(pvv, p1_sb[:, 128 * j:128 * (j + 1)],
                                 p2_sb[:, 512 * j:512 * (j + 1)],
                                 start=(j == 0), stop=(j == 3))
            expkT = mainp.tile([P, 512], bf16)
            nc.scalar.activation(out=expkT, in_=pkv, func=AF.Exp)
            keep_warm(G1, op=expkT[:, 0:128], name="pg1")
            nc.vector.tensor_copy(out=vTo[:, 0:2, 0:128], in_=pvv[:, 0:2, :])
            nc.scalar.activation(out=vTo[:, 2:4, 0:128], in_=pvv[:, 2:4, :],
                                 func=AF.Identity)

            # fused context + softmax denominator per head pair i:
            #   pc_i[:, 0:128] = expkT_i^T @ vT_i ; pc_i[:, 128] = Z
            # Diagonal 64x64 blocks extracted UNSCALED (vector lo / scalar
            # hi); 1/Z rides the A matmul's other operand: wqs_i = rc_i*wq_i.
            pcs = []
            for i in range(4):
                pc = psA.tile([P, 129], fp32, tag="pA", name="pc")
                nc.tensor.matmul(pc, expkT[:, 128 * i:128 * (i + 1)],
                                 vTo[:, i, :], start=True, stop=True)
                pcs.append(pc)

            wo_off = 1024
            A_sb = mainp.tile([P, 4, 256], bf16)
            wqs = mainp.tile([P, 4, 256], bf16)
            wc_sb = mainp.tile([P, 2, 256], bf16)
            pw = [psp.tile([P, 256], fp32, tag="p1", name=f"pw{ck}")
                  for ck in range(2)]
            for i in range(4):
                rc = workp.tile([P, 1], fp32, tag=f"r{i}")
                nc.vector.reciprocal(rc, pcs[i][:, 128:129])
                nc.vector.tensor_scalar_mul(wqs[:, i, :],
                                            p3_sb[:, 256 * i:256 * (i + 1)], rc)
                nc.vector.tensor_copy(out=ctx_bd[0:64, i, 0:64],
                                      in_=pcs[i][0:64, 0:64])
                nc.scalar.activation(out=ctx_bd[64:128, i, 64:128],
                                     in_=pcs[i][64:128, 64:128],
                                     func=AF.Identity)
                pa = psA.tile([P, 256], fp32, tag="pA", name="pa")
                nc.tensor.matmul(pa, ctx_bd[:, i, :], wqs[:, i, :],
                                 start=True, stop=True)
                if i % 2 == 0:
                    nc.vector.tensor_copy(out=A_sb[:, i, :], in_=pa)
                else:
                    nc.scalar.activation(out=A_sb[:, i, :], in_=pa,
                                         func=AF.Identity)
                # W_combT[c, o'] += A[he_i, c] * WoutT_perm[he_i, o']
                for ck in range(2):
                    nc.tensor.matmul(pw[ck], A_sb[:, i, 128 * ck:128 * (ck + 1)],
                                     p3_sb[:, wo_off + 256 * i:wo_off + 256 * (i + 1)],
                                     start=(i == 0), stop=(i == 3))
                if i == 1:
                    keep_warm(G2, op=A_sb[:, 1, 0:128], name="pg2")
            keep_warm(G3, op=A_sb[:, 3, 0:128], name="pg3")
            nc.vector.tensor_copy(out=wc_sb[:, 0, :], in_=pw[0])
            nc.scalar.activation(out=wc_sb[:, 1, :], in_=pw[1],
                                 func=AF.Identity)

            # --- phase 2: OUT = W_comb @ xf + bias, streamed over x chunks.
            # The last spatial tile is split in half so the final
            # compute->store->drain tail is shorter.
            tiles = [(nt, 0, 512) for nt in range(7)]
            tiles += [(7, 0, 256), (7, 256, 256)]
            for nt, c0, cw in tiles:
                cc, sub = nt // 2, nt % 2
                ot = outp.tile([P, 2, 512], bf16, tag="osb")
                for mo in range(2):
                    po = psO.tile([P, 512], fp32, tag="O", name="po")
                    for ck in range(2):
                        nc.tensor.matmul(
                            po[:, 0:cw], wc_sb[:, ck, 128 * mo:128 * (mo + 1)],
                            x_sb[cc][:, ck, 512 * sub + c0:512 * sub + c0 + cw],
                            start=(ck == 0), stop=(ck == 1))
                    if mo == 0:
                        nc.scalar.activation(out=ot[:, mo, 0:cw], in_=po[:, 0:cw],
                                             func=AF.Identity,
                                             bias=bias_sb[:, 0:1], scale=1.0)
                    else:
                        nc.vector.tensor_scalar_add(out=ot[:, mo, 0:cw],
                                                    in0=po[:, 0:cw],
                                                    scalar1=bias_sb[:, 1:2])
                nc.sync.dma_start(out_t[:, nt, :, c0:c0 + cw], ot[:, :, 0:cw])

    nc.compile()
    return nc


def kernel(x, cond_emb, Wq, Wcond, Wout, b_out):
    import ml_dtypes
    from concourse.bass_utils import run_bass_kernel_spmd

    global LAST_RESULTS

    if "nc" not in _CACHE:
        _CACHE["nc"] = _build_nc()
    nc = _CACHE["nc"]

    bf = ml_dtypes.bfloat16

    # cf chunks: cf[j*128+p, m] -> [p, j*128+m]
    cf = np.asarray(cond_emb, np.float32).reshape(B, 4, P, P)
    cf_p = np.transpose(cf, (0, 2, 1, 3)).reshape(B, P, 512)
    # wcondT chunks: wct[j*128+p, o] -> [p, j, o]
    wct = np.ascontiguousarray(np.asarray(Wcond, np.float32).T).reshape(4, P, 1024)
    wck = np.transpose(wct[:, :, 0:512], (1, 0, 2)).reshape(P, 2048)
    wcv = np.transpose(wct[:, :, 512:1024], (1, 0, 2)).reshape(P, 2048)
    # Wq with columns parity-permuted (c = 2j+ck -> block ck, col j)
    wq_perm = np.asarray(Wq, np.float32).reshape(512, P, 2)
    wq_perm = np.transpose(wq_perm, (0, 2, 1)).reshape(512, 256)
    wq_p = np.transpose(wq_perm.reshape(4, P, 256), (1, 0, 2)).reshape(P, 1024)
    # WoutT with columns parity-permuted (o = 2i+mo -> block mo, col i)
    woT = np.ascontiguousarray(np.asarray(Wout, np.float32).T)
    woT_perm = np.transpose(woT.reshape(512, P, 2), (0, 2, 1)).reshape(512, 256)
    wo_p = np.transpose(woT_perm.reshape(4, P, 256), (1, 0, 2)).reshape(P, 1024)

    p1 = np.empty((B, P, 2560), bf)
    p1[:, :, 0:512] = cf_p.astype(bf)
    p1[:, :, 512:2560] = wck[None].astype(bf)
    p2 = np.broadcast_to(wcv.astype(bf), (B, P, 2048))
    p3 = np.empty((P, 2048), bf)
    p3[:, 0:1024] = wq_p.astype(bf)
    p3[:, 1024:2048] = wo_p.astype(bf)
    p3 = np.broadcast_to(p3, (B, P, 2048))
    # x interleaved: xp[p, cc, ck, u] = x[b, 2p+ck, cc*1024+u]
    xr = np.asarray(x, np.float32).reshape(B, P, 2, 4, 1024)
    xp = np.transpose(xr, (0, 1, 3, 2, 4)).astype(bf)
    bias2 = np.ascontiguousarray(
        np.asarray(b_out, np.float32).reshape(P, 2))

    in_maps = [
        {
            "p1": np.ascontiguousarray(p1[b]),
            "p2": np.ascontiguousarray(p2[b]),
            "p3": np.ascontiguousarray(p3[b]),
            "xp": np.ascontiguousarray(xp[b]),
            "bias2": bias2,
        }
        for b in range(B)
    ]

    trace = bool(int(os.environ.get("KERNEL_TRACE", "0")))
    res = run_bass_kernel_spmd(nc, in_maps, core_ids=list(range(N_CORES)),
                               trace=trace)
    LAST_RESULTS = res
    # out_p[p, nt, mo, u] = out[b, 2p+mo, nt*512+u]
    outs = []
    for b in range(B):
        arr = np.asarray(res.results[b]["out"]).astype(np.float32)
        outs.append(np.transpose(arr, (0, 2, 1, 3)).reshape(C, N_SPATIAL))
    return np.stack(outs).reshape(B, C, 64, 64)


if __name__ == "__main__":
    xs = np.random.RandomState(0)
    ins = {
        "x": xs.randn(8, 256, 64, 64).astype(np.float32),
        "cond_emb": xs.randn(8, 512, 1, 128).astype(np.float32),
        "Wq": (xs.randn(512, 256) * 0.05).astype(np.float32),
        "Wcond": (xs.randn(1024, 512) * 0.05).astype(np.float32),
        "Wout": (xs.randn(256, 512) * 0.05).astype(np.float32),
        "b_out": np.zeros(256, np.float32),
    }
    o = kernel(**ins)
    print("ran, shape", o.shape)


# revision 23
# speedup vs baseline: 1.0917x; 1.0917x over previous
"""Trainium2 Bass kernel for ConditionalLinearAttention (v6).

Math (per batch element b, shapes hardcoded):
  xf  = x[b].reshape(256, 4096)
  cf  = cond_emb[b].reshape(512, 128)
  kv  = Wcond @ cf                      # (1024, 128)
  k   = softmax(kv[:512], per-row over the 128 cond positions)
  v   = kv[512:]
  ctx[h] = k_h @ v_h.T                  # (64, 64) per head h
  out = Wout @ apply(ctx) @ Wq @ xf + b_out

ctx is tiny and per-batch, so the whole attention folds into one per-batch
matrix W_comb = Wout @ ctxE @ Wq (256x256); the spatial dimension then sees
ONE (256x256)@(256x4096) GEMM. Sharding: data-parallel over batch, one
batch element per core.

v6 design notes (lessons from v2-v5 ntff profiles):
  * critical path = weight stream -> phase-1 chain -> W_comb -> 7.7us of
    phase-2 matmuls -> out stream; everything else overlaps it.
  * cross-engine handoffs cost 0.3-1us each and DMA completion semaphores
    trail the last byte by 0.7-2us, so: few large weight transfers (one
    semaphore each, v2-proven shapes), the k-half matmuls gated on the
    FIRST transfer only, and the 1/Z softmax scale folded into per-pair Wq
    slices (rc indexes the contraction dim of the A matmul, so it can
    scale either operand) - the diagonal-block extraction then runs as
    plain copies in parallel with the reciprocal.
  * W_comb accumulation is interleaved with A production, reusing the dead
    kv PSUM slots, so W_comb trails the last A chunk by one matmul.
  * the PE HAM clock gate needs ~3.4us of sustained activity to reach
    2.4GHz and re-throttles after idle windows.  Junk matmuls (N=128,
    ~107ns) cover the head; later groups are ANCHORED on phase-1 tiles
    (expkT, A_sb) so the Tile scheduler cannot hoist them early - they
    become ready exactly when the gap they must bridge opens.
  * SWDGE (gpsimd) input DMA was tried and abandoned: its completion sem
    trails data by ~1.5us/transfer (engine-7/15 ring contention).  int8
    x with cast-during-DMA was tried and abandoned: SDMA paces on the
    2-byte SBUF side, so it saves no stream time and costs accuracy.
"""

import os

import numpy as np

B = 8
C = 256
N_SPATIAL = 4096  # 64*64
P = 128
N_CORES = 8

WARM = int(os.environ.get("KERNEL_WARM", "34"))  # PE warmup matmuls at head
G1 = int(os.environ.get("KERNEL_G1", "6"))   # anchored on expkT
G2 = int(os.environ.get("KERNEL_G2", "4"))   # anchored on A_sb[:, 1]
G3 = int(os.environ.get("KERNEL_G3", "8"))   # anchored on A_sb[:, 3]

_CACHE = {}
LAST_RESULTS = None  # BassKernelResults of the most recent run (for test.py)


def _build_nc():
    import concourse.bacc as bacc
    import concourse.mybir as mybir
    import concourse.tile as tile

    fp32 = mybir.dt.float32
    bf16 = mybir.dt.bfloat16
    AF = mybir.ActivationFunctionType

    nc = bacc.Bacc("TRN2", target_bir_lowering=False, debug=False,
                   num_devices=N_CORES)

    # p1:    cf (4x128 cols) + wck (4x512)                  -> [128, 2560]
    # p2:    wcv (4x512)                                    -> [128, 2048]
    # p3:    wq_perm (4x256) + woT_perm (4x256)             -> [128, 2048]
    # xp:    x interleaved c=2p+ck, 4 col-chunks of 1024    -> [128,4,2,1024]
    # bias2: b_out[2p+mo]                                   -> [128, 2] f32
    # outp:  out rows 2p+mo, 8 col-chunks of 512            -> [128,8,2,512]
    p1_t = nc.dram_tensor("p1", [P, 2560], bf16, kind="ExternalInput").ap()
    p2_t = nc.dram_tensor("p2", [P, 2048], bf16, kind="ExternalInput").ap()
    p3_t = nc.dram_tensor("p3", [P, 2048], bf16, kind="ExternalInput").ap()
    xp_t = nc.dram_tensor("xp", [P, 4, 2, 1024], bf16, kind="ExternalInput").ap()
    bias_t = nc.dram_tensor("bias2", [P, 2], fp32, kind="ExternalInput").ap()
    out_t = nc.dram_tensor("out", [P, 8, 2, 512], bf16, kind="ExternalOutput").ap()

    with tile.TileContext(nc) as tc:
        with (
            tc.tile_pool(name="main", bufs=1) as mainp,
            tc.tile_pool(name="work", bufs=2) as workp,
            tc.tile_pool(name="outp", bufs=6) as outp,
            tc.tile_pool(name="ps", bufs=2, space="PSUM") as psp,
            tc.tile_pool(name="psA", bufs=3, space="PSUM") as psA,
            tc.tile_pool(name="psO", bufs=3, space="PSUM") as psO,
        ):
            # --- junk-matmul operand, first so the PE can warm immediately
            wl = mainp.tile([P, 128], bf16)
            nc.vector.memset(wl, 0.0)

            # --- input DMA triggers, critical-path order, all on sync HWDGE
            p1_sb = mainp.tile([P, 2560], bf16)
            nc.sync.dma_start(p1_sb, p1_t)
            p2_sb = mainp.tile([P, 2048], bf16)
            nc.sync.dma_start(p2_sb, p2_t)
            p3_sb = mainp.tile([P, 2048], bf16)
            nc.sync.dma_start(p3_sb, p3_t)
            x_sb = []
            for cc in range(4):
                t = mainp.tile([P, 2, 1024], bf16, tag=f"x{cc}")
                nc.sync.dma_start(t, xp_t[:, cc, :, :])
                x_sb.append(t)
            # small stuff off the sync ring
            bias_sb = mainp.tile([P, 2], fp32)
            nc.gpsimd.dma_start(bias_sb, bias_t)

            # persistent SBUF tiles + fills (not on gpsimd)
            vTo = mainp.tile([P, 4, 129], bf16)
            nc.vector.memset(vTo[:, :, 128:129], 1.0)
            ctx_bd = mainp.tile([P, 4, 128], bf16)
            nc.vector.memset(ctx_bd, 0.0)

            def keep_warm(n, op=None, name="pj"):
                src = op if op is not None else wl
                for _ in range(n):
                    pj = psO.tile([P, 512], fp32, tag="O", name=name)
                    nc.tensor.matmul(pj[:, 0:128], src, src, start=True,
                                     stop=True)

            keep_warm(WARM)

            # --- phase 1: per-batch W_comb (256x256) ---
            # kvT k-half (gated on p1's single semaphore), then v-half
            # (gated on p2) while exp() runs on the scalar engine.
            pkv = psp.tile([P, 512], fp32, tag="p1")
            for j in range(4):
                nc.tensor.matmul(pkv, p1_sb[:, 128 * j:128 * (j + 1)],
                                 p1_sb[:, 512 + 512 * j:512 + 512 * (j + 1)],
                                 start=(j == 0), stop=(j == 3))
            pvv = psp.tile([P, 4, 128], fp32, tag="p1")
            for j in range(4):
                nc.tensor.matmul(pvv, p1_sb[:, 128 * j:128 * (j + 1)],
                                 p2_sb[:, 512 * j:512 * (j + 1)],
                                 start=(j == 0), stop=(j == 3))
            expkT = mainp.tile([P, 512], bf16)
            nc.scalar.activation(out=expkT, in_=pkv, func=AF.Exp)
            keep_warm(G1, op=expkT[:, 0:128], name="pg1")
            nc.vector.tensor_copy(out=vTo[:, 0:2, 0:128], in_=pvv[:, 0:2, :])
            nc.scalar.activation(out=vTo[:, 2:4, 0:128], in_=pvv[:, 2:4, :],
                                 func=AF.Identity)

            # fused context + softmax denominator per head pair i:
            #   pc_i[:, 0:128] = expkT_i^T @ vT_i ; pc_i[:, 128] = Z
            # Diagonal 64x64 blocks extracted UNSCALED (vector lo / scalar
            # hi); 1/Z rides the A matmul's other operand: wqs_i = rc_i*wq_i.
            pcs = []
            for i in range(4):
                pc = psA.tile([P, 129], fp32, tag="pA", name="pc")
                nc.tensor.matmul(pc, expkT[:, 128 * i:128 * (i + 1)],
                                 vTo[:, i, :], start=True, stop=True)
                pcs.append(pc)

            wo_off = 1024
            A_sb = mainp.tile([P, 4, 256], bf16)
            wc_sb = mainp.tile([P, 2, 256], bf16)
            pw = [psp.tile([P, 256], fp32, tag="p1", name=f"pw{ck}")
                  for ck in range(2)]
            for i in range(4):
                rc = workp.tile([P, 1], fp32, tag=f"r{i}")
                nc.vector.reciprocal(rc, pcs[i][:, 128:129])
                nc.vector.tensor_scalar_mul(ctx_bd[0:64, i, 0:64],
                                            pcs[i][0:64, 0:64], rc[0:64])
                nc.scalar.activation(out=ctx_bd[64:128, i, 64:128],
                                     in_=pcs[i][64:128, 64:128],
                                     func=AF.Identity, scale=rc[64:128])
                pa = psA.tile([P, 256], fp32, tag="pA", name="pa")
                nc.tensor.matmul(pa, ctx_bd[:, i, :],
                                 p3_sb[:, 256 * i:256 * (i + 1)],
                                 start=True, stop=True)
                if i % 2 == 0:
                    nc.vector.tensor_copy(out=A_sb[:, i, :], in_=pa)
                else:
                    nc.scalar.activation(out=A_sb[:, i, :], in_=pa,
                                         func=AF.Identity)
                # W_combT[c, o'] += A[he_i, c] * WoutT_perm[he_i, o']
                for ck in range(2):
                    nc.tensor.matmul(pw[ck], A_sb[:, i, 128 * ck:128 * (ck + 1)],
                                     p3_sb[:, wo_off + 256 * i:wo_off + 256 * (i + 1)],
                                     start=(i == 0), stop=(i == 3))
                if i == 1:
                    keep_warm(G2, op=A_sb[:, 1, 0:128], name="pg2")
            keep_warm(G3, op=A_sb[:, 3, 0:128], name="pg3")
            nc.vector.tensor_copy(out=wc_sb[:, 0, :], in_=pw[0])
            nc.scalar.activation(out=wc_sb[:, 1, :], in_=pw[1],
                                 func=AF.Identity)

            # --- phase 2: OUT = W_comb @ xf + bias, streamed over x chunks.
            # The last spatial tile is split in half so the final
            # compute->store->drain tail is shorter.
            tiles = [(nt, 0, 512) for nt in range(7)]
            tiles += [(7, 0, 256), (7, 256, 256)]
            for nt, c0, cw in tiles:
                cc, sub = nt // 2, nt % 2
                ot = outp.tile([P, 2, 512], bf16, tag="osb")
                for mo in range(2):
                    po = psO.tile([P, 512], fp32, tag="O", name="po")
                    for ck in range(2):
                        nc.tensor.matmul(
                            po[:, 0:cw], wc_sb[:, ck, 128 * mo:128 * (mo + 1)],
                            x_sb[cc][:, ck, 512 * sub + c0:512 * sub + c0 + cw],
                            start=(ck == 0), stop=(ck == 1))
                    if mo == 0:
                        nc.scalar.activation(out=ot[:, mo, 0:cw], in_=po[:, 0:cw],
                                             func=AF.Identity,
                                             bias=bias_sb[:, 0:1], scale=1.0)
                    else:
                        nc.vector.tensor_scalar_add(out=ot[:, mo, 0:cw],
                                                    in0=po[:, 0:cw],
                                                    scalar1=bias_sb[:, 1:2])
                nc.sync.dma_start(out_t[:, nt, :, c0:c0 + cw], ot[:, :, 0:cw])

    nc.compile()
    return nc


def kernel(x, cond_emb, Wq, Wcond, Wout, b_out):
    import ml_dtypes
    from concourse.bass_utils import run_bass_kernel_spmd

    global LAST_RESULTS

    if "nc" not in _CACHE:
        _CACHE["nc"] = _build_nc()
    nc = _CACHE["nc"]

    bf = ml_dtypes.bfloat16

    # cf chunks: cf[j*128+p, m] -> [p, j*128+m]
    cf = np.asarray(cond_emb, np.float32).reshape(B, 4, P, P)
    cf_p = np.transpose(cf, (0, 2, 1, 3)).reshape(B, P, 512)
    # wcondT chunks: wct[j*128+p, o] -> [p, j, o]
    wct = np.ascontiguousarray(np.asarray(Wcond, np.float32).T).reshape(4, P, 1024)
    wck = np.transpose(wct[:, :, 0:512], (1, 0, 2)).reshape(P, 2048)
    wcv = np.transpose(wct[:, :, 512:1024], (1, 0, 2)).reshape(P, 2048)
    # Wq with columns parity-permuted (c = 2j+ck -> block ck, col j)
    wq_perm = np.asarray(Wq, np.float32).reshape(512, P, 2)
    wq_perm = np.transpose(wq_perm, (0, 2, 1)).reshape(512, 256)
    wq_p = np.transpose(wq_perm.reshape(4, P, 256), (1, 0, 2)).reshape(P, 1024)
    # WoutT with columns parity-permuted (o = 2i+mo -> block mo, col i)
    woT = np.ascontiguousarray(np.asarray(Wout, np.float32).T)
    woT_perm = np.transpose(woT.reshape(512, P, 2), (0, 2, 1)).reshape(512, 256)
    wo_p = np.transpose(woT_perm.reshape(4, P, 256), (1, 0, 2)).reshape(P, 1024)

    p1 = np.empty((B, P, 2560), bf)
    p1[:, :, 0:512] = cf_p.astype(bf)
    p1[:, :, 512:2560] = wck[None].astype(bf)
    p2 = np.broadcast_to(wcv.astype(bf), (B, P, 2048))
    p3 = np.empty((P, 2048), bf)
    p3[:, 0:1024] = wq_p.astype(bf)
    p3[:, 1024:2048] = wo_p.astype(bf)
    p3 = np.broadcast_to(p3, (B, P, 2048))
    # x interleaved: xp[p, cc, ck, u] = x[b, 2p+ck, cc*1024+u]
    xr = np.asarray(x, np.float32).reshape(B, P, 2, 4, 1024)
    xp = np.transpose(xr, (0, 1, 3, 2, 4)).astype(bf)
    bias2 = np.ascontiguousarray(
        np.asarray(b_out, np.float32).reshape(P, 2))

    in_maps = [
        {
            "p1": np.ascontiguousarray(p1[b]),
            "p2": np.ascontiguousarray(p2[b]),
            "p3": np.ascontiguousarray(p3[b]),
            "xp": np.ascontiguousarray(xp[b]),
            "bias2": bias2,
        }
        for b in range(B)
    ]

    trace = bool(int(os.environ.get("KERNEL_TRACE", "0")))
    res = run_bass_kernel_spmd(nc, in_maps, core_ids=list(range(N_CORES)),
                               trace=trace)
    LAST_RESULTS = res
    # out_p[p, nt, mo, u] = out[b, 2p+mo, nt*512+u]
    outs = []
    for b in range(B):
        arr = np.asarray(res.results[b]["out"]).astype(np.float32)
        outs.append(np.transpose(arr, (0, 2, 1, 3)).reshape(C, N_SPATIAL))
    return np.stack(outs).reshape(B, C, 64, 64)


if __name__ == "__main__":
    xs = np.random.RandomState(0)
    ins = {
        "x": xs.randn(8, 256, 64, 64).astype(np.float32),
        "cond_emb": xs.randn(8, 512, 1, 128).astype(np.float32),
        "Wq": (xs.randn(512, 256) * 0.05).astype(np.float32),
        "Wcond": (xs.randn(1024, 512) * 0.05).astype(np.float32),
        "Wout": (xs.randn(256, 512) * 0.05).astype(np.float32),
        "b_out": np.zeros(256, np.float32),
    }
    o = kernel(**ins)
    print("ran, shape", o.shape)
